# revision 17
# baseline (speedup 1.0000x reference)
"""DDiT block (adaLN-modulated transformer block) on 8 Trainium2 NeuronCores.

Sharding: tokens are split 8 ways (2 batches x 4 sequence chunks of 512
tokens). Activations are kept feature-major ([feature, token]) on-chip so
every matmul contracts over the partition axis without transposes. k/v are
all-gathered within each batch group of 4 cores; adaLN modulation rows are
computed 1/4 per core within each batch group and all-gathered.

Matmuls run in bf16 (fp32 accumulation in PSUM); layernorm statistics,
softmax and residuals stay fp32.
"""
import os
import sys

for _p in ("/opt/trn_rl_repo", "/root/.axon_site/_ro/trn_rl_repo"):
    if os.path.isdir(_p) and _p not in sys.path:
        sys.path.append(_p)

import numpy as np
import ml_dtypes

import concourse.bass as bass
import concourse.mybir as mybir
import concourse.tile as tile
from concourse.bass_utils import run_bass_kernel_spmd
from concourse.vector_clock import ScopedClock

BF16 = ml_dtypes.bfloat16
F32 = np.float32

B, S, H, NH, D, Fd = 2, 2048, 768, 12, 64, 3072
P = 128
NCORES = 8
TOK = S // 4            # 512 tokens per core
KT = H // P             # 6 feature tiles of H
FT = Fd // P            # 24 feature tiles of F
ADA4 = 6 * H            # 4608 ada rows, computed fully on every core
KTA = KT + 1            # 7 contraction tiles (last one carries the bias row)
NCH = ADA4 // 512       # 9 chunks of 512 ada rows
VW = D + 1              # 65: v plus a ones column (softmax denominator)
EPS = 1e-5

AF = mybir.ActivationFunctionType
ALU = mybir.AluOpType
DT = mybir.dt

RG_BATCH = [[0, 1, 2, 3], [4, 5, 6, 7]]


def _patch_tile_drain():
    """The walrus build here allows at most one semaphore wait on SP
    control instructions; TileContext's exit drain attaches several.
    Split them one-per-NOP."""
    if getattr(tile.TileContext, "_ant_drain_patched", False):
        return

    def _split_multiwaits(nc):
        """Rewrite every instruction carrying >1 semaphore wait: excess
        waits move to same-engine NOPs inserted just before it."""
        count = 0
        for f in nc.m.functions:
            for bb in f.blocks:
                insts = bb.instructions
                new = []
                for ins in insts:
                    si = getattr(ins, "sync_info", None)
                    if si is not None and si.on_wait and len(si.on_wait) > 1:
                        waits = list(si.on_wait)
                        si.on_wait = [waits[-1]]
                        for w in waits[:-1]:
                            count += 1
                            nop = mybir.InstNoOp(
                                name=f"antw_{count}_{ins.name}",
                                ins=[], outs=[])
                            nop.engine = ins.engine
                            nop.sync_info = mybir.SyncInfo(
                                on_update=[], on_wait=[w])
                            nc.register_instruction(nop, overwrite=True)
                            new.append(nop)
                    new.append(ins)
                bb.instructions = new

    def _drain_and_barrier(self, tick_clock, wait_clock):
        _split_multiwaits(self.nc)
        drain_inst = self.nc.sync.drain()
        wait_clock.add_sem_waits(
            drain_inst.ins, ScopedClock({None: tick_clock.global_clock})
        )
        si = drain_inst.ins.sync_info
        waits = list(si.on_wait)
        si.on_wait = []
        for w in waits:
            nop = self.nc.sync.nop(nofuse=True, hint="drain_extra_waits")
            nop.ins.sync_info = mybir.SyncInfo(on_update=[], on_wait=[w])
        self.nc.all_engine_barrier()
        popped = self.nc._tile_sem_poison_stack.pop()
        assert popped is self._sem_poison
        self.nc.clear_and_free_semaphores(list(self.sems.allocated().values()))
        self.nc.all_engine_barrier()

    tile.TileContext._drain_and_barrier = _drain_and_barrier
    tile.TileContext._ant_drain_patched = True


def build():
    _patch_tile_drain()
    nc = bass.Bass(num_devices=NCORES)

    def din(name, shape, dt):
        return nc.dram_tensor(name, shape, dt, kind="ExternalInput")

    xT = din("xT", [KT, P, TOK], DT.float32)
    qkvw = din("qkvw", [KT, P, 3 * H], DT.bfloat16)
    attnw = din("attnw", [D, NH * H], DT.bfloat16)
    w1 = din("w1", [KT, P, Fd], DT.bfloat16)
    w2 = din("w2", [FT, P, H], DT.bfloat16)
    b1 = din("b1", [P, FT], DT.float32)
    b2 = din("b2", [P, KT], DT.float32)
    n1w = din("n1w", [P, KT], DT.float32)
    n2w = din("n2w", [P, KT], DT.float32)
    cosf = din("cosf", [P, TOK], DT.bfloat16)
    sinf = din("sinf", [P, TOK], DT.bfloat16)
    adaw = din("adaw", [NCH, P, KTA, 512], DT.bfloat16)
    cT = din("cT", [P, KTA], DT.bfloat16)
    rotp = din("rotp", [P, P], DT.bfloat16)
    onesf = din("onesf", [P, P], DT.float32)

    outT = nc.dram_tensor("outT", [KT, P, TOK], DT.float32,
                          kind="ExternalOutput")

    with tile.TileContext(nc) as tc:
        with tc.tile_pool(name="sb", bufs=1) as sb, \
             tc.tile_pool(name="ps", bufs=1, space="PSUM") as ps, \
             tc.tile_pool(name="dr", bufs=1, space="DRAM") as dr:
            _body(nc, sb, ps, dr, locals())
    return nc


def _body(nc, sb, ps, dr, t):
    xT, qkvw, attnw, w1, w2 = t["xT"], t["qkvw"], t["attnw"], t["w1"], t["w2"]
    b1, b2, n1w, n2w = t["b1"], t["b2"], t["n1w"], t["n2w"]
    cosf, sinf, adaw = t["cosf"], t["sinf"], t["adaw"]
    cT, rotp, onesf, outT = t["cT"], t["rotp"], t["onesf"], t["outT"]

    # ================= constant / weight loads =====================
    zero_c = sb.tile([P, 1], DT.float32)
    nc.vector.memset(zero_c[:], 0.0)
    nc.const_aps.aps[(DT.float32, 0.0)] = zero_c[:]
    eps_c = sb.tile([P, 1], DT.float32)
    nc.vector.memset(eps_c[:], EPS)
    nc.const_aps.aps[(DT.float32, EPS)] = eps_c[:]

    # highest-priority loads: x (layernorm stats) and the adaLN chain
    cT_sb = sb.tile([P, KTA], DT.bfloat16)
    nc.sync.dma_start(cT_sb[:], cT[:])

    x_sb = sb.tile([P, KT, TOK], DT.float32)
    for k in range(KT):
        nc.sync.dma_start(x_sb[:, k, :], xT[k])

    # k-projection weights next (they gate the first all-gather)
    qkvw_sb = sb.tile([P, KT, 3 * H], DT.bfloat16, tag="wbig")
    for k in range(KT):
        nc.sync.dma_start(qkvw_sb[:, k, H:2 * H], qkvw[k][:, H:2 * H])

    ones_sb = sb.tile([P, P], DT.float32)
    nc.sync.dma_start(ones_sb[:], onesf[:])
    rotp_sb = sb.tile([P, P], DT.bfloat16)
    nc.sync.dma_start(rotp_sb[:], rotp[:])
    cos_sb = sb.tile([P, TOK], DT.bfloat16)
    nc.sync.dma_start(cos_sb[:], cosf[:])
    sin_sb = sb.tile([P, TOK], DT.bfloat16)
    nc.sync.dma_start(sin_sb[:], sinf[:])
    n1w_sb = sb.tile([P, KT], DT.float32)
    nc.sync.dma_start(n1w_sb[:], n1w[:])
    n2w_sb = sb.tile([P, KT], DT.float32)
    nc.sync.dma_start(n2w_sb[:], n2w[:])

    # v then q weight columns, then the mlp biases
    for k in range(KT):
        nc.sync.dma_start(qkvw_sb[:, k, 2 * H:3 * H], qkvw[k][:, 2 * H:3 * H])
    for k in range(KT):
        nc.sync.dma_start(qkvw_sb[:, k, 0:H], qkvw[k][:, 0:H])
    b1_sb = sb.tile([P, FT], DT.float32)
    nc.sync.dma_start(b1_sb[:], b1[:])
    b2_sb = sb.tile([P, KT], DT.float32)
    nc.sync.dma_start(b2_sb[:], b2[:])

    # ================= adaLN modulation ============================
    # Every core computes all 4608 modulation rows for its batch locally
    # (GEMV streamed in 9 chunks); the bias rides along as an extra
    # contraction tile whose c-entry is 1. No collective needed.
    mods_dr = dr.tile([NCH, 512], DT.float32)
    for ch in range(NCH):
        at = sb.tile([P, KTA, 512], DT.bfloat16, tag="adat", bufs=2,
                     name=f"adat{ch}")
        nc.sync.dma_start(at[:], adaw[ch])
        pm = ps.tile([1, 512], DT.float32, tag="uno", bufs=4,
                     name=f"ada_{ch}")
        for k in range(KTA):
            nc.tensor.matmul(pm[:], cT_sb[:, k:k + 1], at[:, k, :],
                             start=(k == 0), stop=(k == KTA - 1))
        mstage = sb.tile([1, 512], DT.float32, tag="mstage", bufs=2,
                         name=f"mstage{ch}")
        nc.scalar.copy(mstage[:], pm[:])
        nc.sync.dma_start(mods_dr[ch:ch + 1, :], mstage[:])
    mods_sb = sb.tile([P, 36], DT.float32)
    nc.sync.dma_start(
        mods_sb[:],
        mods_dr[:].rearrange("c s -> (c s)").rearrange("(j p) -> p j", p=P))

    def mod_col(block, tt):
        # 0 shift_msa 1 scale_msa 2 gate_msa 3 shift_mlp 4 scale_mlp 5 gate_mlp
        return mods_sb[:, block * KT + tt:block * KT + tt + 1]

    # A = normw * (1 + scale) per feature tile
    A_msa = sb.tile([P, KT], DT.float32)
    A_mlp = sb.tile([P, KT], DT.float32)
    for tt in range(KT):
        tmp1 = sb.tile([P, 1], DT.float32, tag="tiny", bufs=2, name="tmp1")
        nc.vector.tensor_scalar(tmp1[:], mod_col(1, tt), 1.0, None, ALU.add)
        nc.vector.tensor_tensor(A_msa[:, tt:tt + 1], tmp1[:],
                                n1w_sb[:, tt:tt + 1], ALU.mult)
    for tt in range(KT):
        tmp2 = sb.tile([P, 1], DT.float32, tag="tiny", bufs=2, name="tmp2")
        nc.vector.tensor_scalar(tmp2[:], mod_col(4, tt), 1.0, None, ALU.add)
        nc.vector.tensor_tensor(A_mlp[:, tt:tt + 1], tmp2[:],
                                n2w_sb[:, tt:tt + 1], ALU.mult)

    f32s = dict(tag="f32s", bufs=3)

    def layer_norm(src_sb, A_tile, shift_block, xm_out):
        """src [128,KT,TOK] f32 -> xm_out [128,KT,TOK] bf16 (modulated)."""
        s_ps = ps.tile([1, TOK], DT.float32, tag="uno", bufs=4, name="s_ps")
        q_ps = ps.tile([1, TOK], DT.float32, tag="uno", bufs=4, name="q_ps")
        for tt in range(KT):
            xsq = sb.tile([P, TOK], DT.float32, **f32s, name="xsq")
            nc.vector.tensor_tensor(xsq[:], src_sb[:, tt, :],
                                    src_sb[:, tt, :], ALU.mult)
            nc.tensor.matmul(s_ps[:], ones_sb[:, 0:1], src_sb[:, tt, :],
                             start=(tt == 0), stop=(tt == KT - 1))
            nc.tensor.matmul(q_ps[:], ones_sb[:, 0:1], xsq[:],
                             start=(tt == 0), stop=(tt == KT - 1))
        sa = sb.tile([1, TOK], DT.float32, tag="st_a", bufs=1, name="sa")
        sb2 = sb.tile([1, TOK], DT.float32, tag="st_b", bufs=1, name="sb2")
        sc_ = sb.tile([1, TOK], DT.float32, tag="st_c", bufs=1, name="sc_")
        nc.vector.tensor_scalar(sa[:], s_ps[:], 1.0 / H, None, ALU.mult)
        nc.vector.tensor_scalar(sb2[:], q_ps[:], 1.0 / H, None, ALU.mult)
        nc.vector.tensor_tensor(sc_[:], sa[:], sa[:], ALU.mult)
        nc.vector.tensor_tensor(sb2[:], sb2[:], sc_[:], ALU.subtract)
        nc.scalar.activation(sc_[:], sb2[:], AF.Sqrt, bias=EPS)
        nc.vector.reciprocal(sb2[:], sc_[:])   # sb2 = rstd
        nc.vector.tensor_tensor(sa[:], sa[:], sb2[:], ALU.mult)  # sa = m*rstd
        rstd_ps = ps.tile([P, TOK], DT.float32, tag="uno", bufs=4,
                          name="rstd_ps")
        mr_ps = ps.tile([P, TOK], DT.float32, tag="uno", bufs=4,
                        name="mr_ps")
        nc.tensor.matmul(rstd_ps[:], ones_sb[0:1, :], sb2[:],
                         start=True, stop=True)
        nc.tensor.matmul(mr_ps[:], ones_sb[0:1, :], sa[:],
                         start=True, stop=True)
        for tt in range(KT):
            t1 = sb.tile([P, TOK], DT.float32, **f32s, name="t1")
            nc.vector.tensor_tensor(t1[:], src_sb[:, tt, :], rstd_ps[:],
                                    ALU.mult)
            nc.vector.tensor_tensor(t1[:], t1[:], mr_ps[:], ALU.subtract)
            nc.vector.tensor_scalar(
                xm_out[:, tt, :], t1[:], A_tile[:, tt:tt + 1],
                mod_col(shift_block, tt), ALU.mult, ALU.add)

    # ================= LN1 + qkv ===================================
    xm_sb = sb.tile([P, KT, TOK], DT.bfloat16, tag="xm")
    layer_norm(x_sb, A_msa, 0, xm_sb)

    kaginA = dr.tile([3, P, TOK], DT.bfloat16)
    kagoutA = dr.tile([4, 3, P, TOK], DT.bfloat16)
    vaginA = dr.tile([2, P, NH * VW], DT.bfloat16)
    vagoutA = dr.tile([4, 2, P, NH * VW], DT.bfloat16)
    kaginB = dr.tile([3, P, TOK], DT.bfloat16)
    kagoutB = dr.tile([4, 3, P, TOK], DT.bfloat16)
    vaginB = dr.tile([2, P, NH * VW], DT.bfloat16)
    vagoutB = dr.tile([4, 2, P, NH * VW], DT.bfloat16)

    def qk_tile(m, dest, dest2=None):
        """Feature tile m of the qkv projection + rotary. For q tiles the
        two head halves go to different zero-padded buffers."""
        acc = ps.tile([P, TOK], DT.float32, tag="uno", bufs=4, name="qk_acc")
        for k in range(KT):
            nc.tensor.matmul(acc[:], qkvw_sb[:, k, m * P:(m + 1) * P],
                             xm_sb[:, k, :],
                             start=(k == 0), stop=(k == KT - 1))
        pre = sb.tile([P, TOK], DT.bfloat16, tag="qpre", bufs=2, name="pre")
        nc.scalar.copy(pre[:], acc[:])
        rot = ps.tile([P, TOK], DT.float32, tag="uno", bufs=4, name="rot")
        nc.tensor.matmul(rot[:], rotp_sb[:], pre[:], start=True, stop=True)
        r1 = sb.tile([P, TOK], DT.bfloat16, tag="rr1", bufs=2, name="r1")
        nc.vector.tensor_tensor(r1[:], pre[:], cos_sb[:], ALU.mult)
        r2 = sb.tile([P, TOK], DT.bfloat16, tag="rr2", bufs=2, name="r2")
        nc.vector.tensor_tensor(r2[:], rot[:], sin_sb[:], ALU.mult)
        if dest2 is None:
            nc.vector.tensor_tensor(dest, r1[:], r2[:], ALU.add)
        else:
            nc.vector.tensor_tensor(dest, r1[0:D, :], r2[0:D, :], ALU.add)
            nc.vector.tensor_tensor(dest2, r1[D:P, :], r2[D:P, :], ALU.add)

    vaug_sb = sb.tile([P, 4, NH * VW], DT.bfloat16, tag="vaug")
    for tt in range(4):
        nc.vector.memset(
            vaug_sb[:, tt, :].rearrange("p (h w) -> p h w", w=VW)[:, :, D:D + 1],
            1.0)

    def v_tile(tt):
        for half in range(2):
            acc = ps.tile([P, 6 * D], DT.float32, tag="uno", bufs=4,
                          name="v_acc")
            for k in range(KT):
                nc.tensor.matmul(
                    acc[:], xm_sb[:, k, tt * P:(tt + 1) * P],
                    qkvw_sb[:, k, 2 * H + half * 6 * D:
                            2 * H + (half + 1) * 6 * D],
                    start=(k == 0), stop=(k == KT - 1))
            nc.scalar.copy(
                vaug_sb[:, tt, :]
                .rearrange("p (h w) -> p h w", w=VW)[:, half * 6:(half + 1) * 6, 0:D],
                acc[:].rearrange("p (h d) -> p h d", d=D))

    def ag(in_t, out_t):
        nc.gpsimd.collective_compute(
            "AllGather", ALU.bypass, replica_groups=RG_BATCH,
            ins=[in_t[:].opt()], outs=[out_t[:].opt()])

    # order: kA, vA, kB, vB -- earliest possible start for heads 0-5 and
    # for the first half of every head's attn@v accumulation
    for mm_ in range(3):
        kt_t = sb.tile([P, TOK], DT.bfloat16, tag="ktmp", bufs=2,
                       name=f"ktmp_{mm_}")
        qk_tile(KT + mm_, kt_t[:])
        nc.sync.dma_start(kaginA[mm_], kt_t[:])
    ag(kaginA, kagoutA)
    for tt in range(2):
        v_tile(tt)
        nc.sync.dma_start(vaginA[tt], vaug_sb[:, tt, :])
    ag(vaginA, vagoutA)
    for mm_ in range(3):
        m = 3 + mm_
        kt_t = sb.tile([P, TOK], DT.bfloat16, tag="ktmp", bufs=2,
                       name=f"ktmp_{m}")
        qk_tile(KT + m, kt_t[:])
        nc.sync.dma_start(kaginB[mm_], kt_t[:])
    ag(kaginB, kagoutB)
    for tt in range(2, 4):
        v_tile(tt)
        nc.sync.dma_start(vaginB[tt - 2], vaug_sb[:, tt, :])
    ag(vaginB, vagoutB)

    # q while the all-gathers are in flight. Two zero-padded copies:
    # even heads live in rows 0-63 of qz0 (rows 64-127 zero), odd heads
    # in rows 64-127 of qz1 -- so score matmuls contract the full 128
    # rows (no row-group masking; the partner head's k rows hit zeros).
    qz0_sb = sb.tile([P, KT, TOK], DT.bfloat16, name="qz0")
    qz1_sb = sb.tile([P, KT, TOK], DT.bfloat16, name="qz1")
    nc.vector.memset(qz0_sb[:], 0.0)
    nc.vector.memset(qz1_sb[:], 0.0)
    for m in range(KT):
        qk_tile(m, qz0_sb[0:D, m, :], qz1_sb[D:P, m, :])

    kfull_sb = sb.tile([P, KT, S], DT.bfloat16, tag="chain_a")
    for r in range(4):
        nc.sync.dma_start(kfull_sb[:, 0:3, r * TOK:(r + 1) * TOK],
                          kagoutA[r].rearrange("k p s -> p k s"))
    vfull_sb = sb.tile([P, 16, NH * VW], DT.bfloat16)
    for r in range(4):
        nc.sync.dma_start(vfull_sb[:, 4 * r:4 * r + 2, :],
                          vagoutA[r].rearrange("i p w -> p i w"))
    for r in range(4):
        nc.sync.dma_start(kfull_sb[:, 3:6, r * TOK:(r + 1) * TOK],
                          kagoutB[r].rearrange("k p s -> p k s"))
    for r in range(4):
        nc.sync.dma_start(vfull_sb[:, 4 * r + 2:4 * r + 4, :],
                          vagoutB[r].rearrange("i p w -> p i w"))

    # w1 streams into the big slot once qkv matmuls finish
    w1_sb = sb.tile([P, KT, Fd], DT.bfloat16, tag="wbig")
    for k in range(KT):
        nc.sync.dma_start(w1_sb[:, k, :], w1[k])

    # ================= attention ===================================
    # exp lives in quarter tiles ([P,4,TOK], 6 slots) so consecutive
    # heads overlap and the PE never starves behind ACT's exp stream.
    o_sb = sb.tile([D, NH, TOK], DT.bfloat16, tag="osb")
    # chunk order: kj pairs from the vA token halves first (positions 0,1
    # within each rank block), then the vB halves -- matches AG arrival
    CHUNKS = [(0, 1), (4, 5), (8, 9), (12, 13),
              (2, 3), (6, 7), (10, 11), (14, 15)]
    o_tiles = {}
    rd_tiles = {}

    def emit_scores(h, ci, eq):
        ht = h // 2
        qz = qz0_sb if h % 2 == 0 else qz1_sb
        sc = ps.tile([P, 2 * TOK], DT.float32, tag="duo", bufs=2,
                     name=f"sc_{h}_{ci}")
        for i, kj in enumerate(CHUNKS[ci]):
            nc.tensor.matmul(
                sc[:, i * TOK:(i + 1) * TOK],
                kfull_sb[:, ht, kj * P:(kj + 1) * P],
                qz[:, ht, :],
                start=True, stop=True)
        nc.scalar.activation(
            eq[:, (ci % 2) * 2:(ci % 2) * 2 + 2, :].rearrange(
                "p a s -> p (a s)"),
            sc[:], AF.Exp, scale=0.125)

    def emit_av(h, ci, eq):
        o_ps = o_tiles[h]
        for i, kj in enumerate(CHUNKS[ci]):
            nc.tensor.matmul(
                o_ps[:], vfull_sb[:, kj, h * VW:(h + 1) * VW],
                eq[:, (ci % 2) * 2 + i, :],
                start=(ci == 0 and i == 0), stop=(ci == 7 and i == 1))

    def emit_norm(h):
        o_ps = o_tiles.pop(h)
        rd = rd_tiles.pop(h)
        rdb_ps = ps.tile([D, TOK], DT.float32, tag="uno", bufs=4,
                         name=f"rdb_{h}")
        nc.tensor.matmul(rdb_ps[:], ones_sb[D:D + 1, 0:D], rd[D:D + 1, :],
                         start=True, stop=True, tile_position=(D, 0))
        rdc = sb.tile([D, TOK], DT.float32, tag="rdc", bufs=2,
                      name=f"rdc_{h}")
        nc.vector.tensor_copy(rdc[:], rdb_ps[:])
        nc.vector.tensor_tensor(o_sb[:, h, :], o_ps[0:D, :], rdc[:],
                                ALU.mult)

    eq_map = {}
    for h in range(NH):
        o_tiles[h] = ps.tile([VW, TOK], DT.float32, tag="uno", bufs=4,
                             name=f"o_ps_{h}")
        for ci in range(8):
            if ci % 2 == 0:
                eq_map[(h, ci // 2)] = sb.tile(
                    [P, 4, TOK], DT.bfloat16, tag="scr4", bufs=6,
                    name=f"exp_{h}_{ci // 2}")
            eq = eq_map[(h, ci // 2)]
            emit_scores(h, ci, eq)
            if ci > 0:
                emit_av(h, ci - 1, eq_map[(h, (ci - 1) // 2)])
            if ci == 4 and h > 0:
                emit_norm(h - 1)
        emit_av(h, 7, eq_map[(h, 3)])
        rd = sb.tile([P, TOK], DT.float32, tag="rd", bufs=2, name=f"rd_{h}")
        rd_tiles[h] = rd
        nc.vector.reciprocal(rd[D:D + 1, :], o_tiles[h][D:D + 1, :])
        for q_ in range(4):
            eq_map.pop((h, q_), None)
    emit_norm(NH - 1)

    # ================= attn_out + residual =========================
    attnw_sb = sb.tile([D, NH * H], DT.bfloat16, tag="chain_a")
    nc.sync.dma_start(attnw_sb[:], attnw[:])
    for m in range(KT):
        acc = ps.tile([P, TOK], DT.float32, tag="uno", bufs=4,
                      name=f"ao_{m}")
        for h in range(NH):
            nc.tensor.matmul(
                acc[:], attnw_sb[:, h * H + m * P:h * H + (m + 1) * P],
                o_sb[:, h, :], start=(h == 0), stop=(h == NH - 1))
        tg = sb.tile([P, TOK], DT.float32, **f32s, name="tg")
        nc.vector.tensor_scalar(tg[:], acc[:], mod_col(2, m), None, ALU.mult)
        nc.vector.tensor_tensor(x_sb[:, m, :], tg[:], x_sb[:, m, :], ALU.add)

    # ================= LN2 + MLP ===================================
    xm2_sb = sb.tile([P, KT, TOK], DT.bfloat16, tag="xm", name="xm2")
    layer_norm(x_sb, A_mlp, 3, xm2_sb)

    hdn_tiles = []
    for g in range(KT):
        hq = sb.tile([P, 4, TOK], DT.bfloat16, tag="scr4", bufs=6,
                     name=f"hdn_{g}")
        hdn_tiles.append(hq)
        for r in range(4):
            m = g * 4 + r
            acc = ps.tile([P, TOK], DT.float32, tag="uno", bufs=4,
                          name=f"m1_{m}")
            for k in range(KT):
                nc.tensor.matmul(acc[:], w1_sb[:, k, m * P:(m + 1) * P],
                                 xm2_sb[:, k, :],
                                 start=(k == 0), stop=(k == KT - 1))
            nc.scalar.activation(hq[:, r, :], acc[:], AF.Gelu_apprx_tanh,
                                 bias=b1_sb[:, m:m + 1])

    w2_sb = sb.tile([P, FT, H], DT.bfloat16, tag="wbig")
    for k in range(FT):
        nc.sync.dma_start(w2_sb[:, k, :], w2[k])

    for m in range(KT):
        acc = ps.tile([P, TOK], DT.float32, tag="uno", bufs=4,
                      name=f"m2_{m}")
        for k in range(FT):
            nc.tensor.matmul(acc[:], w2_sb[:, k, m * P:(m + 1) * P],
                             hdn_tiles[k // 4][:, k % 4, :],
                             start=(k == 0), stop=(k == FT - 1))
        tg = sb.tile([P, TOK], DT.float32, **f32s, name="tg2")
        nc.vector.tensor_scalar(tg[:], acc[:], b2_sb[:, m:m + 1],
                                mod_col(5, m), ALU.add, ALU.mult)
        nc.vector.tensor_tensor(x_sb[:, m, :], tg[:], x_sb[:, m, :], ALU.add)
        nc.sync.dma_start(outT[m], x_sb[:, m, :])


_CACHE = {}


def _get_nc():
    if "nc" not in _CACHE:
        _CACHE["nc"] = build()
    return _CACHE["nc"]


def _rot_perm():
    blk = np.zeros((D, D), F32)
    for i in range(32):
        blk[i, i + 32] = 1.0
    for i in range(32, D):
        blk[i, i - 32] = -1.0
    out = np.zeros((P, P), F32)
    out[0:D, 0:D] = blk
    out[D:P, D:P] = blk
    return out


def _prep_core_inputs(inputs, core):
    b, j = divmod(core, 4)
    sl = slice(j * TOK, (j + 1) * TOK)
    x = np.asarray(inputs["x"], F32)
    qkv_w = np.asarray(inputs["qkv_w"], F32)
    attn_out_w = np.asarray(inputs["attn_out_w"], F32)
    mlp_w1 = np.asarray(inputs["mlp_w1"], F32)
    mlp_w2 = np.asarray(inputs["mlp_w2"], F32)
    ada_w = np.asarray(inputs["ada_w"], F32)
    ada_b = np.asarray(inputs["ada_b"], F32)
    cc = np.asarray(inputs["c"], F32)
    cos = np.asarray(inputs["cos"], F32)
    sin = np.asarray(inputs["sin"], F32)

    def fm(vec):  # [n*128] -> [128, n] feature-major
        return np.ascontiguousarray(vec.reshape(-1, P).T, dtype=F32)

    d = {}
    d["xT"] = np.ascontiguousarray(x[b, sl].T).reshape(KT, P, TOK)
    d["qkvw"] = np.ascontiguousarray(qkv_w.T).reshape(KT, P, 3 * H).astype(BF16)
    d["attnw"] = np.ascontiguousarray(
        attn_out_w.T.reshape(NH, D, H).transpose(1, 0, 2).reshape(D, NH * H)
    ).astype(BF16)
    d["w1"] = np.ascontiguousarray(mlp_w1.T).reshape(KT, P, Fd).astype(BF16)
    d["w2"] = np.ascontiguousarray(mlp_w2.T).reshape(FT, P, H).astype(BF16)
    d["b1"] = fm(np.asarray(inputs["mlp_b1"], F32))
    d["b2"] = fm(np.asarray(inputs["mlp_b2"], F32))
    d["n1w"] = fm(np.asarray(inputs["norm1_w"], F32))
    d["n2w"] = fm(np.asarray(inputs["norm2_w"], F32))
    cosT = np.ascontiguousarray(cos[0, sl, 0, 0, :].T)  # [64, 512]
    sinT = np.ascontiguousarray(sin[0, sl, 0, 0, :].T)
    d["cosf"] = np.vstack([cosT, cosT]).astype(BF16)
    d["sinf"] = np.vstack([sinT, sinT]).astype(BF16)
    # adaw: [NCH, KTA, P, 512]; contraction tile KT holds the bias in
    # row 0 (matched by cT's extra column with a single 1).
    aw = np.ascontiguousarray(ada_w.T).reshape(KT, P, ADA4)
    awx = np.zeros((KTA, P, ADA4), F32)
    awx[:KT] = aw
    awx[KT, 0, :] = ada_b
    d["adaw"] = np.ascontiguousarray(
        awx.reshape(KTA, P, NCH, 512).transpose(2, 1, 0, 3)).astype(BF16)
    ctx = np.zeros((P, KTA), F32)
    ctx[:, :KT] = fm(cc[b])
    ctx[0, KT] = 1.0
    d["cT"] = ctx.astype(BF16)
    d["rotp"] = _rot_perm().astype(BF16)
    d["onesf"] = np.ones((P, P), F32)
    return d


def kernel(**inputs):
    nc = _get_nc()
    in_maps = [_prep_core_inputs(inputs, c) for c in range(NCORES)]
    res = run_bass_kernel_spmd(nc, in_maps, core_ids=list(range(NCORES)))
    out = np.empty((B, S, H), F32)
    for core in range(NCORES):
        b, j = divmod(core, 4)
        o = res.results[core]["outT"].reshape(H, TOK)
        out[b, j * TOK:(j + 1) * TOK, :] = o.T
    return out



# revision 23
# speedup vs baseline: 1.5170x; 1.5170x over previous
"""DDiT block (adaLN-modulated transformer block) on 8 Trainium2 NeuronCores.

Sharding: tokens are split 8 ways (2 batches x 4 sequence chunks of 512
tokens). Activations are kept feature-major ([feature, token]) on-chip so
every matmul contracts over the partition axis without transposes. k/v are
all-gathered within each batch group of 4 cores (two combined fp8
all-gathers, one per half); adaLN modulation rows are computed fully on
every core (no collective), split into an early pass (shift/scale_msa,
needed by LN1) and a late pass (gates + mlp mods).

Matmuls run in fp8e4m3 with DoubleRow pairing where the contraction is
>=256 (fp32 accumulation in PSUM); layernorm statistics, softmax and
residuals stay fp32. Weights are scaled x64 and activations x8 host/chip
side; the scales are folded into existing activation/gate multiplies.
"""
import os
import sys

for _p in ("/opt/trn_rl_repo", "/root/.axon_site/_ro/trn_rl_repo"):
    if os.path.isdir(_p) and _p not in sys.path:
        sys.path.append(_p)

import numpy as np
import ml_dtypes

import concourse.bass as bass
import concourse.mybir as mybir
import concourse.tile as tile
from concourse.bass_utils import run_bass_kernel_spmd
from concourse.vector_clock import ScopedClock

BF16 = ml_dtypes.bfloat16
F8 = ml_dtypes.float8_e4m3
F32 = np.float32

B, S, H, NH, D, Fd = 2, 2048, 768, 12, 64, 3072
P = 128
NCORES = 8
TOK = S // 4            # 512 tokens per core
KT = H // P             # 6 feature tiles of H
FT = Fd // P            # 24 feature tiles of F
ADA4 = 6 * H            # 4608 ada rows, computed fully on every core
KTA = KT + 1            # 7 contraction tiles (last one carries the bias row)
NCH = ADA4 // 512       # 9 chunks of 512 ada rows
VW = D + 1              # 65: v plus a ones column (softmax denominator)
VROW = NH * VW + 4      # 784: padded to a 16-byte multiple for DoubleRow
EPS = 1e-5
WS = 64.0               # fp8 weight scale
AS = 8.0                # fp8 activation scale (q/k/v/o and c)

AF = mybir.ActivationFunctionType
ALU = mybir.AluOpType
DT = mybir.dt
DRM = mybir.MatmulPerfMode.DoubleRow

RG_BATCH = [[0, 1, 2, 3], [4, 5, 6, 7]]

KN = 3 * P * TOK            # fp8 elements of one k half
VN = 2 * P * VROW           # fp8 elements of one v half


def _patch_tile_drain():
    """The walrus build here allows at most one semaphore wait on SP
    control instructions; TileContext's exit drain attaches several.
    Split them one-per-NOP."""
    if getattr(tile.TileContext, "_ant_drain_patched", False):
        return

    def _split_multiwaits(nc):
        """Rewrite every instruction carrying >1 semaphore wait: excess
        waits move to same-engine NOPs inserted just before it."""
        count = 0
        for f in nc.m.functions:
            for bb in f.blocks:
                insts = bb.instructions
                new = []
                for ins in insts:
                    si = getattr(ins, "sync_info", None)
                    if si is not None and si.on_wait and len(si.on_wait) > 1:
                        waits = list(si.on_wait)
                        si.on_wait = [waits[-1]]
                        for w in waits[:-1]:
                            count += 1
                            nop = mybir.InstNoOp(
                                name=f"antw_{count}_{ins.name}",
                                ins=[], outs=[])
                            nop.engine = ins.engine
                            nop.sync_info = mybir.SyncInfo(
                                on_update=[], on_wait=[w])
                            nc.register_instruction(nop, overwrite=True)
                            new.append(nop)
                    new.append(ins)
                bb.instructions = new

    def _drain_and_barrier(self, tick_clock, wait_clock):
        _split_multiwaits(self.nc)
        drain_inst = self.nc.sync.drain()
        wait_clock.add_sem_waits(
            drain_inst.ins, ScopedClock({None: tick_clock.global_clock})
        )
        si = drain_inst.ins.sync_info
        waits = list(si.on_wait)
        si.on_wait = []
        for w in waits:
            nop = self.nc.sync.nop(nofuse=True, hint="drain_extra_waits")
            nop.ins.sync_info = mybir.SyncInfo(on_update=[], on_wait=[w])
        self.nc.all_engine_barrier()
        popped = self.nc._tile_sem_poison_stack.pop()
        assert popped is self._sem_poison
        self.nc.clear_and_free_semaphores(list(self.sems.allocated().values()))
        self.nc.all_engine_barrier()

    tile.TileContext._drain_and_barrier = _drain_and_barrier
    tile.TileContext._ant_drain_patched = True


def build():
    _patch_tile_drain()
    nc = bass.Bass(num_devices=NCORES)

    def din(name, shape, dt):
        return nc.dram_tensor(name, shape, dt, kind="ExternalInput")

    xT = din("xT", [KT, P, TOK], DT.float32)
    qkvw = din("qkvw", [KT, P, 3 * H], DT.float8e4)
    attnw = din("attnw", [P, KT, H], DT.float8e4)
    w1 = din("w1", [KT, P, Fd], DT.float8e4)
    w2 = din("w2", [FT, P, H], DT.float8e4)
    b1 = din("b1", [P, FT], DT.float32)
    b2 = din("b2", [P, KT], DT.float32)
    n1w = din("n1w", [P, KT], DT.float32)
    n2w = din("n2w", [P, KT], DT.float32)
    cosf = din("cosf", [P, TOK], DT.bfloat16)
    sinf = din("sinf", [P, TOK], DT.bfloat16)
    adaw = din("adaw", [NCH, P, KTA, 512], DT.bfloat16)
    cT = din("cT", [P, KTA], DT.bfloat16)
    rotp = din("rotp", [P, P], DT.float8e4)
    onesf = din("onesf", [P, P], DT.float32)

    outT = nc.dram_tensor("outT", [KT, P, TOK], DT.float32,
                          kind="ExternalOutput")

    with tile.TileContext(nc) as tc:
        with tc.tile_pool(name="sb", bufs=1) as sb, \
             tc.tile_pool(name="ps", bufs=1, space="PSUM") as ps, \
             tc.tile_pool(name="dr", bufs=1, space="DRAM") as dr:
            _body(nc, sb, ps, dr, locals())
    return nc


def _body(nc, sb, ps, dr, t):
    xT, qkvw, attnw, w1, w2 = t["xT"], t["qkvw"], t["attnw"], t["w1"], t["w2"]
    b1, b2, n1w, n2w = t["b1"], t["b2"], t["n1w"], t["n2w"]
    cosf, sinf, adaw = t["cosf"], t["sinf"], t["adaw"]
    cT, rotp, onesf, outT = t["cT"], t["rotp"], t["onesf"], t["outT"]

    # ================= constant / weight loads =====================
    zero_c = sb.tile([P, 1], DT.float32)
    nc.vector.memset(zero_c[:], 0.0)
    nc.const_aps.aps[(DT.float32, 0.0)] = zero_c[:]
    eps_c = sb.tile([P, 1], DT.float32)
    nc.vector.memset(eps_c[:], EPS)
    nc.const_aps.aps[(DT.float32, EPS)] = eps_c[:]

    cT_sb = sb.tile([P, KTA], DT.bfloat16)
    nc.sync.dma_start(cT_sb[:], cT[:])

    x_sb = sb.tile([P, KT, TOK], DT.float32)
    for k in range(KT):
        nc.sync.dma_start(x_sb[:, k, :], xT[k])

    # k-projection weights next (they gate the first all-gather)
    qkvw_sb = sb.tile([P, KT, 3 * H], DT.float8e4, tag="wbig")
    for k in range(KT):
        nc.sync.dma_start(qkvw_sb[:, k, H:2 * H], qkvw[k][:, H:2 * H])

    ones_sb = sb.tile([P, P], DT.float32)
    nc.sync.dma_start(ones_sb[:], onesf[:])
    rotp_sb = sb.tile([P, P], DT.float8e4)
    nc.sync.dma_start(rotp_sb[:], rotp[:])
    cos_sb = sb.tile([P, TOK], DT.bfloat16)
    nc.sync.dma_start(cos_sb[:], cosf[:])
    sin_sb = sb.tile([P, TOK], DT.bfloat16)
    nc.sync.dma_start(sin_sb[:], sinf[:])
    n1w_sb = sb.tile([P, KT], DT.float32)
    nc.sync.dma_start(n1w_sb[:], n1w[:])

    for k in range(KT):
        nc.sync.dma_start(qkvw_sb[:, k, 2 * H:3 * H], qkvw[k][:, 2 * H:3 * H])
    for k in range(KT):
        nc.sync.dma_start(qkvw_sb[:, k, 0:H], qkvw[k][:, 0:H])
    n2w_sb = sb.tile([P, KT], DT.float32)
    nc.sync.dma_start(n2w_sb[:], n2w[:])
    b1_sb = sb.tile([P, FT], DT.float32)
    nc.sync.dma_start(b1_sb[:], b1[:])
    b2_sb = sb.tile([P, KT], DT.float32)
    nc.sync.dma_start(b2_sb[:], b2[:])

    # ================= adaLN modulation ============================
    # GEMV over fp8 adaw (x64) with cT (x8); bias rides in tile KT with
    # a c-entry of 8 -> PSUM holds 512*mods, rescaled on the PSUM copy.
    mods_sb = sb.tile([P, 36], DT.float32)

    def ada_pass(chunks, stage_dr):
        for i, ch in enumerate(chunks):
            at = sb.tile([P, KTA, 512], DT.bfloat16, tag="adat", bufs=2,
                         name=f"adat{ch}")
            nc.sync.dma_start(at[:], adaw[ch])
            pm = ps.tile([1, 512], DT.float32, tag="uno", bufs=4,
                         name=f"ada_{ch}")
            for k in range(KTA):
                nc.tensor.matmul(pm[:], cT_sb[:, k:k + 1], at[:, k, :],
                                 start=(k == 0), stop=(k == KTA - 1))
            mstage = sb.tile([1, 512], DT.float32, tag="mstage", bufs=2,
                             name=f"mstage{ch}")
            nc.scalar.copy(mstage[:], pm[:])
            nc.sync.dma_start(stage_dr[i:i + 1, :], mstage[:])

    m1_dr = dr.tile([3, 512], DT.float32)
    ada_pass(range(3), m1_dr)
    nc.sync.dma_start(
        mods_sb[:, 0:12],
        m1_dr[:].rearrange("c s -> (c s)").rearrange("(j p) -> p j", p=P))

    def mod_col(block, tt):
        # 0 shift_msa 1 scale_msa 2 gate_msa 3 shift_mlp 4 scale_mlp 5 gate_mlp
        return mods_sb[:, block * KT + tt:block * KT + tt + 1]

    # A = normw * (1 + scale) per feature tile
    A_msa = sb.tile([P, KT], DT.float32)
    A_mlp = sb.tile([P, KT], DT.float32)
    for tt in range(KT):
        tmp1 = sb.tile([P, 1], DT.float32, tag="tiny", bufs=2, name="tmp1")
        nc.vector.tensor_scalar(tmp1[:], mod_col(1, tt), 1.0, None, ALU.add)
        nc.vector.tensor_tensor(A_msa[:, tt:tt + 1], tmp1[:],
                                n1w_sb[:, tt:tt + 1], ALU.mult)

    f32s = dict(tag="f32s", bufs=3)

    def layer_norm(src_sb, A_tile, shift_block, xm_out):
        """src [128,KT,TOK] f32 -> xm_out [128,KT,TOK] fp8 (modulated)."""
        s_ps = ps.tile([1, TOK], DT.float32, tag="uno", bufs=4, name="s_ps")
        q_ps = ps.tile([1, TOK], DT.float32, tag="uno", bufs=4, name="q_ps")
        for tt in range(KT):
            xsq = sb.tile([P, TOK], DT.float32, **f32s, name="xsq")
            nc.vector.tensor_tensor(xsq[:], src_sb[:, tt, :],
                                    src_sb[:, tt, :], ALU.mult)
            nc.tensor.matmul(s_ps[:], ones_sb[:, 0:1], src_sb[:, tt, :],
                             start=(tt == 0), stop=(tt == KT - 1))
            nc.tensor.matmul(q_ps[:], ones_sb[:, 0:1], xsq[:],
                             start=(tt == 0), stop=(tt == KT - 1))
        sa = sb.tile([1, TOK], DT.float32, tag="st_a", bufs=1, name="sa")
        sb2 = sb.tile([1, TOK], DT.float32, tag="st_b", bufs=1, name="sb2")
        sc_ = sb.tile([1, TOK], DT.float32, tag="st_c", bufs=1, name="sc_")
        nc.vector.tensor_scalar(sa[:], s_ps[:], 1.0 / H, None, ALU.mult)
        nc.vector.tensor_scalar(sb2[:], q_ps[:], 1.0 / H, None, ALU.mult)
        nc.vector.tensor_tensor(sc_[:], sa[:], sa[:], ALU.mult)
        nc.vector.tensor_tensor(sb2[:], sb2[:], sc_[:], ALU.subtract)
        nc.scalar.activation(sc_[:], sb2[:], AF.Sqrt, bias=EPS)
        nc.vector.reciprocal(sb2[:], sc_[:])   # sb2 = rstd
        nc.vector.tensor_tensor(sa[:], sa[:], sb2[:], ALU.mult)  # sa = m*rstd
        rstd_ps = ps.tile([P, TOK], DT.float32, tag="uno", bufs=4,
                          name="rstd_ps")
        mr_ps = ps.tile([P, TOK], DT.float32, tag="uno", bufs=4,
                        name="mr_ps")
        nc.tensor.matmul(rstd_ps[:], ones_sb[0:1, :], sb2[:],
                         start=True, stop=True)
        nc.tensor.matmul(mr_ps[:], ones_sb[0:1, :], sa[:],
                         start=True, stop=True)
        for tt in range(KT):
            t1 = sb.tile([P, TOK], DT.float32, **f32s, name="t1")
            nc.vector.tensor_tensor(t1[:], src_sb[:, tt, :], rstd_ps[:],
                                    ALU.mult)
            nc.vector.tensor_tensor(t1[:], t1[:], mr_ps[:], ALU.subtract)
            nc.vector.tensor_scalar(
                xm_out[:, tt, :], t1[:], A_tile[:, tt:tt + 1],
                mod_col(shift_block, tt), ALU.mult, ALU.add)

    # ================= LN1 + qkv ===================================
    xm_sb = sb.tile([P, KT, TOK], DT.float8e4, tag="xm")
    layer_norm(x_sb, A_msa, 0, xm_sb)

    kvaA_in = dr.tile([KN + VN], DT.float8e4)
    kvaA_out = dr.tile([4, KN + VN], DT.float8e4)
    kvaB_in = dr.tile([KN + VN], DT.float8e4)
    kvaB_out = dr.tile([4, KN + VN], DT.float8e4)

    def qk_tile(m, dest, dest2=None):
        """Feature tile m of the qkv projection + rotary. For q tiles the
        two head halves go to different zero-padded buffers."""
        acc = ps.tile([P, TOK], DT.float32, tag="uno", bufs=4, name="qk_acc")
        for kk in range(0, KT, 2):
            nc.tensor.matmul(acc[:], qkvw_sb[:, kk:kk + 2, m * P:(m + 1) * P],
                             xm_sb[:, kk:kk + 2, :],
                             start=(kk == 0), stop=(kk == KT - 2),
                             perf_mode=DRM)
        pre = sb.tile([P, TOK], DT.float8e4, tag="qpre", bufs=2, name="pre")
        nc.scalar.activation(pre[:], acc[:], AF.Copy, scale=AS / WS)
        rot = ps.tile([P, TOK], DT.float32, tag="uno", bufs=4, name="rot")
        nc.tensor.matmul(rot[:], rotp_sb[:], pre[:], start=True, stop=True)
        r1 = sb.tile([P, TOK], DT.bfloat16, tag="rr1", bufs=2, name="r1")
        nc.vector.tensor_tensor(r1[:], pre[:], cos_sb[:], ALU.mult)
        r2 = sb.tile([P, TOK], DT.bfloat16, tag="rr2", bufs=2, name="r2")
        nc.vector.tensor_tensor(r2[:], rot[:], sin_sb[:], ALU.mult)
        if dest2 is None:
            nc.vector.tensor_tensor(dest, r1[:], r2[:], ALU.add)
        else:
            nc.vector.tensor_tensor(dest, r1[0:D, :], r2[0:D, :], ALU.add)
            nc.vector.tensor_tensor(dest2, r1[D:P, :], r2[D:P, :], ALU.add)

    vaug_sb = sb.tile([P, 4, VROW], DT.float8e4, tag="vaug")
    for tt in range(4):
        nc.vector.memset(
            vaug_sb[:, tt, 0:NH * VW].rearrange("p (h w) -> p h w", w=VW)[:, :, D:D + 1],
            1.0)

    def v_tile(tt):
        for half in range(2):
            acc = ps.tile([P, 6 * D], DT.float32, tag="uno", bufs=4,
                          name="v_acc")
            for kk in range(0, KT, 2):
                nc.tensor.matmul(
                    acc[:], xm_sb[:, kk:kk + 2, tt * P:(tt + 1) * P],
                    qkvw_sb[:, kk:kk + 2, 2 * H + half * 6 * D:
                            2 * H + (half + 1) * 6 * D],
                    start=(kk == 0), stop=(kk == KT - 2),
                    perf_mode=DRM)
            nc.scalar.activation(
                vaug_sb[:, tt, 0:NH * VW]
                .rearrange("p (h w) -> p h w", w=VW)[:, half * 6:(half + 1) * 6, 0:D],
                acc[:].rearrange("p (h d) -> p h d", d=D),
                AF.Copy, scale=AS / WS)

    def ag(in_t, out_t):
        nc.gpsimd.collective_compute(
            "AllGather", ALU.bypass, replica_groups=RG_BATCH,
            ins=[in_t[:].opt()], outs=[out_t[:].opt()])

    # half A: k head-pair tiles 0-2 + v token-quarters 0-1; then half B
    for mm_ in range(3):
        kt_t = sb.tile([P, TOK], DT.float8e4, tag="ktmp", bufs=2,
                       name=f"ktmp_{mm_}")
        qk_tile(KT + mm_, kt_t[:])
        nc.sync.dma_start(
            kvaA_in[mm_ * P * TOK:(mm_ + 1) * P * TOK]
            .rearrange("(p s) -> p s", p=P), kt_t[:])
    for tt in range(2):
        v_tile(tt)
        nc.sync.dma_start(
            kvaA_in[KN + tt * P * VROW:KN + (tt + 1) * P * VROW]
            .rearrange("(p w) -> p w", p=P), vaug_sb[:, tt, :])
    ag(kvaA_in, kvaA_out)
    for mm_ in range(3):
        kt_t = sb.tile([P, TOK], DT.float8e4, tag="ktmp", bufs=2,
                       name=f"ktmp_{3 + mm_}")
        qk_tile(KT + 3 + mm_, kt_t[:])
        nc.sync.dma_start(
            kvaB_in[mm_ * P * TOK:(mm_ + 1) * P * TOK]
            .rearrange("(p s) -> p s", p=P), kt_t[:])
    for tt in range(2, 4):
        v_tile(tt)
        nc.sync.dma_start(
            kvaB_in[KN + (tt - 2) * P * VROW:KN + (tt - 1) * P * VROW]
            .rearrange("(p w) -> p w", p=P), vaug_sb[:, tt, :])
    ag(kvaB_in, kvaB_out)

    # ada pass 2 (gates + mlp mods) fills the all-gather window
    m2_dr = dr.tile([6, 512], DT.float32)
    ada_pass(range(3, NCH), m2_dr)
    nc.sync.dma_start(
        mods_sb[:, 12:36],
        m2_dr[:].rearrange("c s -> (c s)").rearrange("(j p) -> p j", p=P))
    for tt in range(KT):
        tmp2 = sb.tile([P, 1], DT.float32, tag="tiny", bufs=2, name="tmp2")
        nc.vector.tensor_scalar(tmp2[:], mod_col(4, tt), 1.0, None, ALU.add)
        nc.vector.tensor_tensor(A_mlp[:, tt:tt + 1], tmp2[:],
                                n2w_sb[:, tt:tt + 1], ALU.mult)
    # gates pre-divided by the fp8 scale products of their consumers
    gsc = sb.tile([P, 2 * KT], DT.float32)
    nc.vector.tensor_scalar(gsc[:, 0:KT], mods_sb[:, 2 * KT:3 * KT],
                            1.0 / (WS * AS), None, ALU.mult)
    nc.vector.tensor_scalar(gsc[:, KT:2 * KT], mods_sb[:, 5 * KT:6 * KT],
                            1.0 / WS, None, ALU.mult)

    # q while the all-gathers are in flight. Two zero-padded copies:
    # even heads live in rows 0-63 of qz0 (rows 64-127 zero), odd heads
    # in rows 64-127 of qz1 -- so score matmuls contract the full 128
    # rows (no row-group masking; the partner head's k rows hit zeros).
    qz0_sb = sb.tile([P, KT, TOK], DT.float8e4, name="qz0")
    qz1_sb = sb.tile([P, KT, TOK], DT.float8e4, name="qz1")
    nc.vector.memset(qz0_sb[:], 0.0)
    nc.vector.memset(qz1_sb[:], 0.0)
    for m in range(KT):
        qk_tile(m, qz0_sb[0:D, m, :], qz1_sb[D:P, m, :])

    kfull_sb = sb.tile([P, KT, S], DT.float8e4, tag="chain_a")
    vfull_sb = sb.tile([P, 16, VROW], DT.float8e4)
    for r in range(4):
        nc.sync.dma_start(
            kfull_sb[:, 0:3, r * TOK:(r + 1) * TOK],
            kvaA_out[r, 0:KN].rearrange("(k p s) -> p k s", p=P, s=TOK))
    for r in range(4):
        nc.sync.dma_start(
            vfull_sb[:, 4 * r:4 * r + 2, :],
            kvaA_out[r, KN:].rearrange("(i p w) -> p i w", p=P, w=VROW))
    for r in range(4):
        nc.sync.dma_start(
            kfull_sb[:, 3:6, r * TOK:(r + 1) * TOK],
            kvaB_out[r, 0:KN].rearrange("(k p s) -> p k s", p=P, s=TOK))
    for r in range(4):
        nc.sync.dma_start(
            vfull_sb[:, 4 * r + 2:4 * r + 4, :],
            kvaB_out[r, KN:].rearrange("(i p w) -> p i w", p=P, w=VROW))

    # w1 streams into the big slot once qkv matmuls finish
    w1_sb = sb.tile([P, KT, Fd], DT.float8e4, tag="wbig")
    for k in range(KT):
        nc.sync.dma_start(w1_sb[:, k, :], w1[k])

    # ================= attention ===================================
    # exp lives in quarter tiles ([P,4,TOK], 6 slots) so consecutive
    # heads overlap and the PE never starves behind ACT's exp stream.
    o_sb = sb.tile([P, KT, TOK], DT.float8e4, tag="osb")
    # chunk order: kj pairs from the vA token halves first (positions 0,1
    # within each rank block), then the vB halves -- matches AG arrival
    CHUNKS = [(0, 1), (4, 5), (8, 9), (12, 13),
              (2, 3), (6, 7), (10, 11), (14, 15)]
    o_tiles = {}
    rd_tiles = {}

    def emit_scores(h, ci, eq):
        ht = h // 2
        qz = qz0_sb if h % 2 == 0 else qz1_sb
        sc = ps.tile([P, 2 * TOK], DT.float32, tag="duo", bufs=2,
                     name=f"sc_{h}_{ci}")
        for i, kj in enumerate(CHUNKS[ci]):
            nc.tensor.matmul(
                sc[:, i * TOK:(i + 1) * TOK],
                kfull_sb[:, ht, kj * P:(kj + 1) * P],
                qz[:, ht, :],
                start=True, stop=True)
        nc.scalar.activation(
            eq[:, (ci % 2) * 2:(ci % 2) * 2 + 2, :].rearrange(
                "p a s -> p (a s)"),
            sc[:], AF.Exp, scale=0.125 / (AS * AS))

    def emit_av(h, ci, eq):
        o_ps = o_tiles[h]
        kj = CHUNKS[ci][0]
        nc.tensor.matmul(
            o_ps[:], vfull_sb[:, kj:kj + 2, h * VW:(h + 1) * VW],
            eq[:, (ci % 2) * 2:(ci % 2) * 2 + 2, :],
            start=(ci == 0), stop=(ci == 7), perf_mode=DRM)

    def emit_norm(h):
        o_ps = o_tiles.pop(h)
        rd = rd_tiles.pop(h)
        rdb_ps = ps.tile([D, TOK], DT.float32, tag="uno", bufs=4,
                         name=f"rdb_{h}")
        nc.tensor.matmul(rdb_ps[:], ones_sb[D:D + 1, 0:D], rd[D:D + 1, :],
                         start=True, stop=True, tile_position=(D, 0))
        rdc = sb.tile([D, TOK], DT.float32, tag="rdc", bufs=2,
                      name=f"rdc_{h}")
        nc.vector.tensor_copy(rdc[:], rdb_ps[:])
        nc.vector.tensor_tensor(
            o_sb[(h % 2) * D:(h % 2) * D + D, h // 2, :],
            o_ps[0:D, :], rdc[:], ALU.mult)

    eq_map = {}
    for h in range(NH):
        o_tiles[h] = ps.tile([VW, TOK], DT.float32, tag="uno", bufs=4,
                             name=f"o_ps_{h}")
        for ci in range(8):
            if ci % 2 == 0:
                eq_map[(h, ci // 2)] = sb.tile(
                    [P, 4, TOK], DT.float8e4, tag="scr4", bufs=6,
                    name=f"exp_{h}_{ci // 2}")
            eq = eq_map[(h, ci // 2)]
            emit_scores(h, ci, eq)
            if ci > 0:
                emit_av(h, ci - 1, eq_map[(h, (ci - 1) // 2)])
            if ci == 4 and h > 0:
                emit_norm(h - 1)
        emit_av(h, 7, eq_map[(h, 3)])
        rd = sb.tile([P, TOK], DT.float32, tag="rd", bufs=2, name=f"rd_{h}")
        rd_tiles[h] = rd
        nc.vector.reciprocal(rd[D:D + 1, :], o_tiles[h][D:D + 1, :])
        for q_ in range(4):
            eq_map.pop((h, q_), None)
    emit_norm(NH - 1)

    # ================= attn_out + residual =========================
    attnw_sb = sb.tile([P, KT, H], DT.float8e4, tag="chain_a")
    nc.sync.dma_start(attnw_sb[:], attnw[:])
    for m in range(KT):
        acc = ps.tile([P, TOK], DT.float32, tag="uno", bufs=4,
                      name=f"ao_{m}")
        for t in range(0, KT, 2):
            nc.tensor.matmul(
                acc[:], attnw_sb[:, t:t + 2, m * P:(m + 1) * P],
                o_sb[:, t:t + 2, :],
                start=(t == 0), stop=(t == KT - 2), perf_mode=DRM)
        tg = sb.tile([P, TOK], DT.float32, **f32s, name="tg")
        nc.vector.tensor_scalar(tg[:], acc[:], gsc[:, m:m + 1], None,
                                ALU.mult)
        nc.vector.tensor_tensor(x_sb[:, m, :], tg[:], x_sb[:, m, :], ALU.add)

    # ================= LN2 + MLP ===================================
    xm2_sb = sb.tile([P, KT, TOK], DT.float8e4, tag="xm", name="xm2")
    layer_norm(x_sb, A_mlp, 3, xm2_sb)

    hdn_tiles = []
    for g in range(KT):
        hq = sb.tile([P, 4, TOK], DT.float8e4, tag="scr4", bufs=6,
                     name=f"hdn_{g}")
        hdn_tiles.append(hq)
        for r in range(4):
            m = g * 4 + r
            acc = ps.tile([P, TOK], DT.float32, tag="uno", bufs=4,
                          name=f"m1_{m}")
            for kk in range(0, KT, 2):
                nc.tensor.matmul(acc[:],
                                 w1_sb[:, kk:kk + 2, m * P:(m + 1) * P],
                                 xm2_sb[:, kk:kk + 2, :],
                                 start=(kk == 0), stop=(kk == KT - 2),
                                 perf_mode=DRM)
            nc.scalar.activation(hq[:, r, :], acc[:], AF.Gelu_apprx_tanh,
                                 bias=b1_sb[:, m:m + 1], scale=1.0 / WS)

    w2_sb = sb.tile([P, FT, H], DT.float8e4, tag="wbig")
    for k in range(FT):
        nc.sync.dma_start(w2_sb[:, k, :], w2[k])

    for m in range(KT):
        acc = ps.tile([P, TOK], DT.float32, tag="uno", bufs=4,
                      name=f"m2_{m}")
        for k in range(0, FT, 2):
            nc.tensor.matmul(acc[:], w2_sb[:, k:k + 2, m * P:(m + 1) * P],
                             hdn_tiles[k // 4][:, k % 4:k % 4 + 2, :],
                             start=(k == 0), stop=(k == FT - 2),
                             perf_mode=DRM)
        tg = sb.tile([P, TOK], DT.float32, **f32s, name="tg2")
        nc.vector.tensor_scalar(tg[:], acc[:], b2_sb[:, m:m + 1],
                                gsc[:, KT + m:KT + m + 1],
                                ALU.add, ALU.mult)
        nc.vector.tensor_tensor(x_sb[:, m, :], tg[:], x_sb[:, m, :], ALU.add)
        nc.sync.dma_start(outT[m], x_sb[:, m, :])


_CACHE = {}


def _get_nc():
    if "nc" not in _CACHE:
        _CACHE["nc"] = build()
    return _CACHE["nc"]


def _rot_perm():
    blk = np.zeros((D, D), F32)
    for i in range(32):
        blk[i, i + 32] = 1.0
    for i in range(32, D):
        blk[i, i - 32] = -1.0
    out = np.zeros((P, P), F32)
    out[0:D, 0:D] = blk
    out[D:P, D:P] = blk
    return out


def _f8(a):
    return np.clip(a, -240.0, 240.0).astype(F8)


def _prep_core_inputs(inputs, core):
    b, j = divmod(core, 4)
    sl = slice(j * TOK, (j + 1) * TOK)
    x = np.asarray(inputs["x"], F32)
    qkv_w = np.asarray(inputs["qkv_w"], F32)
    attn_out_w = np.asarray(inputs["attn_out_w"], F32)
    mlp_w1 = np.asarray(inputs["mlp_w1"], F32)
    mlp_w2 = np.asarray(inputs["mlp_w2"], F32)
    ada_w = np.asarray(inputs["ada_w"], F32)
    ada_b = np.asarray(inputs["ada_b"], F32)
    cc = np.asarray(inputs["c"], F32)
    cos = np.asarray(inputs["cos"], F32)
    sin = np.asarray(inputs["sin"], F32)

    def fm(vec):  # [n*128] -> [128, n] feature-major
        return np.ascontiguousarray(vec.reshape(-1, P).T, dtype=F32)

    d = {}
    d["xT"] = np.ascontiguousarray(x[b, sl].T).reshape(KT, P, TOK)
    d["qkvw"] = _f8(
        np.ascontiguousarray(qkv_w.T).reshape(KT, P, 3 * H) * WS)
    # attn_out: head-pair stacked [128 (2 head feats), KT pair tiles, H]
    a = attn_out_w.T.reshape(NH, D, H)
    d["attnw"] = _f8(
        a.reshape(KT, 2 * D, H).transpose(1, 0, 2) * WS)
    d["w1"] = _f8(np.ascontiguousarray(mlp_w1.T).reshape(KT, P, Fd) * WS)
    d["w2"] = _f8(np.ascontiguousarray(mlp_w2.T).reshape(FT, P, H) * WS)
    d["b1"] = fm(np.asarray(inputs["mlp_b1"], F32))
    d["b2"] = fm(np.asarray(inputs["mlp_b2"], F32)) * WS
    d["n1w"] = fm(np.asarray(inputs["norm1_w"], F32))
    d["n2w"] = fm(np.asarray(inputs["norm2_w"], F32))
    cosT = np.ascontiguousarray(cos[0, sl, 0, 0, :].T)  # [64, 512]
    sinT = np.ascontiguousarray(sin[0, sl, 0, 0, :].T)
    d["cosf"] = np.vstack([cosT, cosT]).astype(BF16)
    d["sinf"] = np.vstack([sinT, sinT]).astype(BF16)
    # adaw: [NCH, P, KTA, 512]; contraction tile KT holds the bias in
    # row 0 (matched by cT's extra column whose entry is AS).
    aw = np.ascontiguousarray(ada_w.T).reshape(KT, P, ADA4)
    awx = np.zeros((KTA, P, ADA4), F32)
    awx[:KT] = aw
    awx[KT, 0, :] = ada_b
    d["adaw"] = np.ascontiguousarray(
        awx.reshape(KTA, P, NCH, 512).transpose(2, 1, 0, 3)).astype(BF16)
    ctx = np.zeros((P, KTA), F32)
    ctx[:, :KT] = fm(cc[b])
    ctx[0, KT] = 1.0
    d["cT"] = ctx.astype(BF16)
    d["rotp"] = _rot_perm().astype(F8)
    d["onesf"] = np.ones((P, P), F32)
    return d


def kernel(**inputs):
    nc = _get_nc()
    in_maps = [_prep_core_inputs(inputs, c) for c in range(NCORES)]
    res = run_bass_kernel_spmd(nc, in_maps, core_ids=list(range(NCORES)))
    out = np.empty((B, S, H), F32)
    for core in range(NCORES):
        b, j = divmod(core, 4)
        o = res.results[core]["outT"].reshape(H, TOK)
        out[b, j * TOK:(j + 1) * TOK, :] = o.T
    return out
